# revision 1
# baseline (speedup 1.0000x reference)
"""Self-contained Trainium2 Bass kernel for the fused attention layer.

Full computation:
    qkv = data @ W_qkv + b_qkv ; split into q,k,v heads (H=16, HD=64)
    scores = softmax(q k^T / sqrt(64)) ; out = (scores @ v) @ W_out + b_out

Sharding over 8 NeuronCores: core c = (batch n = c//2, head-group g = c%2).
Each core computes attention for 8 of the 16 heads of one batch and its
partial output projection; host sums the two partials per batch and adds
b_out.

Per-core pipeline (fp32 data, fp32r matmuls; x^T feature-major layout so no
transposes are ever needed):
  passA: K^T = Wk^T x and V (natural orientation, per-head 64 cols + a ones
         column that yields the softmax denominator Z during the A@V matmul;
         v-bias applied as a rank-1 ones x bv matmul)
  passB: Q^T = Wq^T x  (x streamed from DRAM a second time)
  attention, per (head, 1024-wide q block), key-chunk pipelined:
         S^T psum [128 keys, q] -> exp on ACT (scale=1/8; no max subtraction:
         |scores| < ~3 by construction) -> P^T -> AV^T psum [65, q] row 64 = Z
         normalize: DVE reciprocal of Z row + DMA partition-broadcast + mul
  out-projection consuming AO^T directly; host adds b_out and transposes.

Attention-phase pools are opened BEFORE the projection pools so their
SBUF/PSUM addresses do not overlap: the first scores matmul would otherwise
wait for the projection pools' release.

`reps` emits the whole pipeline N times back to back (used only by the
benchmark harness to wall-clock the steady-state iteration time through the
high-latency axon tunnel).
"""

import sys

for _p in ("/opt/trn_rl_repo",):
    if _p not in sys.path:
        sys.path.insert(0, _p)

import numpy as np

import concourse.bass as bass
import concourse.mybir as mybir
import concourse.tile as tile
from concourse import bacc
from concourse.bass_utils import run_bass_kernel_spmd

F32 = mybir.dt.float32
F32R = mybir.dt.float32r
U32 = mybir.dt.uint32
EXP = mybir.ActivationFunctionType.Exp
P = 128
ONE_F32_BITS = 0x3F800000


def build_nc(L=2048, DIN=1024, HG=8, HD=64, DOUT=1024, reps=1,
             bcast='dma', loop_n=0, interleave_p3=True, norm_bufs=1, pt_bufs=3,
             section='full'):
    """Build the per-core Bass program (SPMD across the 8 cores)."""
    DH = HG * HD               # head-group width (q, k, v each)
    KC = DIN // P              # contraction chunks for the projections
    NKC = L // P               # key chunks
    QHW = min(1024, L)         # q-block width for attention
    NQH = L // QHW
    NW = min(512, L)           # matmul moving width
    NXQ = L // NW              # x^T quarters
    MD = DH // P               # 128-row chunks per of q/k
    SCALE = 1.0 / np.sqrt(HD)
    VW = HG * (HD + 1)         # v tile width (per-head 64 cols + ones col)

    nc = bacc.Bacc("TRN2", target_bir_lowering=False, debug=False)

    xt = nc.dram_tensor("xt", [DIN, L], F32R, kind="ExternalInput")
    wq = nc.dram_tensor("wq", [DIN, DH], F32R, kind="ExternalInput")
    wk = nc.dram_tensor("wk", [DIN, DH], F32R, kind="ExternalInput")
    wv = nc.dram_tensor("wv", [DIN, DH], F32R, kind="ExternalInput")
    bqk = nc.dram_tensor("bqk", [2 * DH], F32, kind="ExternalInput")
    bv = nc.dram_tensor("bv", [1, DH], F32R, kind="ExternalInput")
    wo = nc.dram_tensor("wo", [DH, DOUT], F32R, kind="ExternalInput")
    out_t = nc.dram_tensor("out_t", [DOUT, L], F32, kind="ExternalOutput")
    # tiny pass-through so the benchmark harness has a cheap in/out pair
    tok = nc.dram_tensor("tok", [1, 4], F32, kind="ExternalInput")
    tok_out = nc.dram_tensor("tok_out", [1, 4], F32, kind="ExternalOutput")

    def emit(tc, x):
        """Emit one full pipeline; x is a name prefix."""
        with (
            tc.tile_pool(name=x + "persist", bufs=1) as persist,
            # attention-phase pools: opened before the projection pools so
            # their addresses don't overlap (no release-gating of phase 2)
            tc.tile_pool(name=x + "pt_pool", bufs=pt_bufs) as pt_pool,
            tc.tile_pool(name=x + "rz_pool", bufs=norm_bufs) as rz_pool,
            tc.tile_pool(name=x + "rb_pool", bufs=norm_bufs) as rb_pool,
            tc.tile_pool(name=x + "at_pool", bufs=1) as at_pool,
            tc.tile_pool(name=x + "ps2", bufs=2, space="PSUM") as ps2,
        ):
            # qkt rows: m < MD -> q^T (from wq), m >= MD -> k^T (from wk)
            qkt_sb = [[persist.tile([P, NW], F32R, tag=f"qkt{m}_{q}",
                                    name=f"{x}qkt{m}_{q}")
                       for q in range(NXQ)] for m in range(2 * MD)]
            vt_sb = [persist.tile([P, VW], F32R, tag=f"vt{k}",
                                  name=f"{x}vt{k}") for k in range(NKC)]
            bqk_sb = persist.tile([P, 2 * MD], F32, tag="bqk", name=x + "bqk_sb")
            bv_sb = persist.tile([1, DH], F32R, tag="bv", name=x + "bv_sb")
            ones_sb = persist.tile([1, P], F32R, tag="ones", name=x + "ones_sb")
            onesr_sb = persist.tile([HD + 1, P], F32R, tag="onesr",
                                    name=x + "onesr_sb")

            nc.sync.dma_start(bqk_sb[:], bqk.rearrange("(m p) -> p m", p=P))
            nc.sync.dma_start(bv_sb[:], bv[:])
            nc.vector.memset(ones_sb[:].bitcast(U32), ONE_F32_BITS)
            nc.vector.memset(onesr_sb[:].bitcast(U32), ONE_F32_BITS)
            for k in range(NKC):
                for h in range(HG):
                    c0 = h * (HD + 1) + HD
                    nc.vector.memset(vt_sb[k][:, c0:c0 + 1].bitcast(U32),
                                     ONE_F32_BITS)

            # ---------------- projections (passA: k+v, passB: q) ----------
            with (
                tc.tile_pool(name=x + "xt_pool", bufs=16) as xt_pool,
                tc.tile_pool(name=x + "ps1", bufs=2, space="PSUM") as ps1,
            ):
                wk_sb, wv_sb, wq_sb = [], [], []

                def load_xq(xq, which):
                    xs = slice(xq * NW, (xq + 1) * NW)
                    xts = []
                    for k in range(KC):
                        t = xt_pool.tile([P, NW], F32R, tag="xt",
                                         name=f"{x}xt{which}{xq}_{k}")
                        nc.sync.dma_start(t[:], xt[k * P:(k + 1) * P, xs])
                        xts.append(t)
                    return xts

                def proj_quarter(xq, which, xts=None):
                    if xts is None:
                        xts = load_xq(xq, which)
                    if which == "a":                   # k^T and v
                        for m in range(MD):
                            ps = ps1.tile([P, NW], F32, tag="ps1",
                                          name=f"{x}pk{xq}_{m}")
                            for k in range(KC):
                                nc.tensor.matmul(
                                    ps[:], wk_sb[k][:, m * P:(m + 1) * P],
                                    xts[k][:], start=(k == 0),
                                    stop=(k == KC - 1))
                            nc.vector.tensor_scalar_add(
                                qkt_sb[MD + m][xq][:], ps[:],
                                bqk_sb[:, MD + m:MD + m + 1])
                        for rc in range(NW // P):
                            kr = (xq * NW) // P + rc
                            ps = ps1.tile([P, DH], F32, tag="ps1",
                                          name=f"{x}pv{kr}")
                            for k in range(KC):
                                nc.tensor.matmul(
                                    ps[:], xts[k][:, rc * P:(rc + 1) * P],
                                    wv_sb[k][:], start=(k == 0), stop=False)
                            nc.tensor.matmul(
                                ps[:], ones_sb[:1, :P], bv_sb[:],
                                start=False, stop=True)
                            nc.vector.tensor_copy(
                                vt_sb[kr].rearrange("p (h w) -> p h w",
                                                    w=HD + 1)[:, :, 0:HD],
                                ps.rearrange("p (h w) -> p h w", w=HD))
                    else:                              # q^T
                        for m in range(MD):
                            ps = ps1.tile([P, NW], F32, tag="ps1",
                                          name=f"{x}pq{xq}_{m}")
                            for k in range(KC):
                                nc.tensor.matmul(
                                    ps[:], wq_sb[k][:, m * P:(m + 1) * P],
                                    xts[k][:], start=(k == 0),
                                    stop=(k == KC - 1))
                            nc.vector.tensor_scalar_add(
                                qkt_sb[m][xq][:], ps[:], bqk_sb[:, m:m + 1])

                with (
                    tc.tile_pool(name=x + "wk_pool", bufs=1) as wk_pool,
                    tc.tile_pool(name=x + "wv_pool", bufs=1) as wv_pool,
                ):
                    xts0 = []
                    for k in range(KC):
                        tk = wk_pool.tile([P, DH], F32R, tag=f"wk{k}",
                                          name=f"{x}wk{k}")
                        nc.sync.dma_start(tk[:], wk[k * P:(k + 1) * P, :])
                        wk_sb.append(tk)
                        t = xt_pool.tile([P, NW], F32R, tag="xt",
                                         name=f"{x}xta0_{k}")
                        nc.sync.dma_start(t[:], xt[k * P:(k + 1) * P, 0:NW])
                        xts0.append(t)
                    for k in range(KC):
                        tv = wv_pool.tile([P, DH], F32R, tag=f"wv{k}",
                                          name=f"{x}wv{k}")
                        nc.sync.dma_start(tv[:], wv[k * P:(k + 1) * P, :])
                        wv_sb.append(tv)
                    proj_quarter(0, "a", xts=xts0)
                    for xq in range(1, NXQ):
                        proj_quarter(xq, "a")
                with tc.tile_pool(name=x + "wq_pool", bufs=1) as wq_pool:
                    for k in range(KC):
                        tq = wq_pool.tile([P, DH], F32R, tag=f"wq{k}",
                                          name=f"{x}wq{k}")
                        nc.sync.dma_start(tq[:], wq[k * P:(k + 1) * P, :])
                        wq_sb.append(tq)
                    for xq in range(NXQ):
                        proj_quarter(xq, "b")

            # ---------------- attention + output projection ----------------
            with (
                tc.tile_pool(name=x + "ao_pool", bufs=1) as ao_pool,
                tc.tile_pool(name=x + "wo_pool", bufs=1) as wo_pool,
                tc.tile_pool(name=x + "ot_pool", bufs=4) as ot_pool,
                tc.tile_pool(name=x + "psav", bufs=2, space="PSUM") as psav,
            ):
                ao_sb = [ao_pool.tile([P, L], F32R, tag=f"ao{j}",
                                      name=f"{x}ao{j}") for j in range(MD)]
                wo_sb = [wo_pool.tile([P, DOUT], F32R, tag=f"wo{d}",
                                      name=f"{x}wo{d}") for d in range(MD)]
                for d in range(MD):
                    nc.sync.dma_start(wo_sb[d][:], wo[d * P:(d + 1) * P, :])

                def att(h, qh):
                    j, po = h // 2, HD * (h % 2)
                    hv = slice(h * (HD + 1), (h + 1) * (HD + 1))
                    q0 = qh * QHW
                    av = psav.tile([HD + 1, QHW], F32, tag="av",
                                   name=f"{x}av{h}_{qh}")
                    for kc in range(NKC):
                        sp = ps2.tile([P, QHW], F32, tag="sc",
                                      name=f"{x}sp{h}_{qh}_{kc}")
                        for qs in range(QHW // NW):
                            mq = slice(qs * NW, (qs + 1) * NW)
                            kq, ko = divmod(kc * P, NW)
                            qq = (q0 + qs * NW) // NW
                            nc.tensor.matmul(
                                sp[:, mq],
                                qkt_sb[MD + j][kq][po:po + HD, ko:ko + P],
                                qkt_sb[j][qq][po:po + HD, :],
                                start=True, stop=True)
                        pt = pt_pool.tile([P, QHW], F32R, tag="pt",
                                          name=f"{x}pt{h}_{qh}_{kc}")
                        nc.scalar.activation(pt[:], sp[:], EXP,
                                             scale=float(SCALE))
                        for qs in range(QHW // NW):
                            mq = slice(qs * NW, (qs + 1) * NW)
                            nc.tensor.matmul(
                                av[:, mq], vt_sb[kc][:, hv], pt[:, mq],
                                start=(kc == 0), stop=(kc == NKC - 1))
                    # normalize by Z (= row HD of av): reciprocal of the
                    # psum row, DMA partition-broadcast, multiply
                    rz = rz_pool.tile([HD + 1, QHW], F32R, tag="rz",
                                      name=f"{x}rz{h}_{qh}")
                    with nc.allow_low_precision(reason="1/Z as fp32r"):
                        nc.vector.reciprocal(rz[HD:HD + 1, :],
                                             av[HD:HD + 1, :])
                    rb = rb_pool.tile([HD, QHW], F32, tag="rb",
                                      name=f"{x}rb{h}_{qh}")
                    if bcast == "pool":
                        nc.gpsimd.partition_broadcast(
                            rb[:], rz[HD:HD + 1, :].bitcast(F32))
                    elif bcast == "dma":
                        zrow = rz[HD:HD + 1, :]
                        bsrc = bass.AP(zrow.tensor, zrow.offset,
                                       [[zrow.ap[0][0], 1], [0, HD],
                                        [1, QHW]]).bitcast(F32)
                        nc.sync.dma_start(rb[:], bsrc)
                    else:
                        bc = ps2.tile([HD, QHW], F32, tag="sc",
                                      name=f"{x}bc{h}_{qh}")
                        for qs in range(QHW // NW):
                            mq = slice(qs * NW, (qs + 1) * NW)
                            nc.tensor.matmul(
                                bc[:, mq], onesr_sb[HD:HD + 1, :HD],
                                rz[HD:HD + 1, mq], start=True, stop=True)
                        nc.vector.tensor_copy(rb[:], bc[:])
                    if po == 0:
                        nc.vector.tensor_mul(
                            ao_sb[j][0:HD, q0:q0 + QHW], av[0:HD, :], rb[:])
                    else:
                        at = at_pool.tile([HD, QHW], F32R, tag="at",
                                          name=f"{x}at{h}_{qh}")
                        nc.vector.tensor_mul(at[:], av[0:HD, :], rb[:])
                        nc.sync.dma_start(
                            ao_sb[j][po:po + HD, q0:q0 + QHW], at[:])

                def out_chain(oc, qb):
                    if section == "attn":
                        return
                    qs_ = slice(qb * NW, (qb + 1) * NW)
                    ps = psav.tile([P, NW], F32, tag="av",
                                   name=f"{x}po{oc}_{qb}")
                    for d in range(MD):
                        nc.tensor.matmul(
                            ps[:], wo_sb[d][:, oc * P:(oc + 1) * P],
                            ao_sb[d][:, qs_],
                            start=(d == 0), stop=(d == MD - 1))
                    ot = ot_pool.tile([P, NW], F32, tag="ot",
                                      name=f"{x}ot{oc}_{qb}")
                    nc.vector.tensor_copy(ot[:], ps[:])
                    nc.sync.dma_start(out_t[oc * P:(oc + 1) * P, qs_], ot[:])

                if section == "proj":
                    return
                for h in range(HG):
                    att(h, 0)
                # later q-blocks: optionally interleave the previous blocks'
                # output projection columns between heads
                done_qb = 0
                for qh in range(1, NQH):
                    early = [(oc, qb)
                             for qb in range(done_qb, qh * QHW // NW)
                             for oc in range(DOUT // P)]
                    if interleave_p3:
                        done_qb = qh * QHW // NW
                    per = (len(early) + HG - 1) // HG
                    for h in range(HG):
                        att(h, qh)
                        if interleave_p3:
                            for oc, qb in early[h * per:(h + 1) * per]:
                                out_chain(oc, qb)
                for qb in range(done_qb, NXQ):
                    for oc in range(DOUT // P):
                        out_chain(oc, qb)

    with tile.TileContext(nc) as tc:
        with tc.tile_pool(name="tokp", bufs=1) as tokp:
            tok_sb = tokp.tile([1, 4], F32, tag="tok", name="tok_sb")
            nc.sync.dma_start(tok_sb[:], tok[:])
            nc.sync.dma_start(tok_out[:], tok_sb[:])
        if loop_n:
            with tc.For_i(0, loop_n, 1):
                emit(tc, "")
        else:
            for rep in range(reps):
                emit(tc, f"r{rep}_" if reps > 1 else "")

    nc.compile()
    return nc


_NC_CACHE = {}


def get_nc(**kw):
    key = tuple(sorted(kw.items()))
    if key not in _NC_CACHE:
        _NC_CACHE[key] = build_nc(**kw)
    return _NC_CACHE[key]


def make_in_maps(data, W_qkv, b_qkv, W_out, n_cores=8):
    """Shard full inputs: core c -> (batch c//2, head-group c%2)."""
    data = np.asarray(data, np.float32)
    W_qkv = np.asarray(W_qkv, np.float32)
    b_qkv = np.asarray(b_qkv, np.float32)
    W_out = np.asarray(W_out, np.float32)
    N = data.shape[0]
    DIM = W_out.shape[0]
    G = n_cores // N                      # head groups
    DH = DIM // G
    in_maps = []
    for c in range(n_cores):
        n, g = c // G, c % G
        sq = slice(g * DH, (g + 1) * DH)
        sk = slice(DIM + g * DH, DIM + (g + 1) * DH)
        sv = slice(2 * DIM + g * DH, 2 * DIM + (g + 1) * DH)
        in_maps.append({
            "xt": np.ascontiguousarray(data[n].T),
            "wq": np.ascontiguousarray(W_qkv[:, sq]),
            "wk": np.ascontiguousarray(W_qkv[:, sk]),
            "wv": np.ascontiguousarray(W_qkv[:, sv]),
            "bqk": np.ascontiguousarray(
                np.concatenate([b_qkv[sq], b_qkv[sk]])),
            "bv": np.ascontiguousarray(b_qkv[sv][None, :]),
            "wo": np.ascontiguousarray(W_out[g * DH:(g + 1) * DH, :]),
            "tok": np.zeros((1, 4), np.float32),
        })
    return in_maps


def kernel(data, W_qkv, b_qkv, W_out, b_out):
    data = np.asarray(data, np.float32)
    b_out = np.asarray(b_out, np.float32)
    N, L, DIN = data.shape
    DIM = np.asarray(W_out).shape[0]
    G = 8 // N
    HD = 64
    nc = get_nc(L=L, DIN=DIN, HG=DIM // HD // G, HD=HD, DOUT=DIM)
    in_maps = make_in_maps(data, W_qkv, b_qkv, W_out)
    res = run_bass_kernel_spmd(nc, in_maps, core_ids=list(range(8)))
    out = np.empty((N, L, DIM), np.float32)
    for n in range(N):
        acc = res.results[G * n]["out_t"].copy()
        for g in range(1, G):
            acc += res.results[G * n + g]["out_t"]
        out[n] = acc.T + b_out
    return out

